# revision 63
# baseline (speedup 1.0000x reference)
"""Trainium2 Bass kernel for AtomTransformerBlock (sliding-window attention, W=64).

Sharding: 8 cores x 256 contiguous query atoms (sequence parallel), halo
recompute, no collectives. All heavy compute bf16.

v3: windowed 2-chunk attention (each 128-query half attends to 2x128 w-chunks
instead of 3x256), pair LN stats via DMA-accumulate trees + small DVE tail,
z-transpose via DMA xbar (off the PE), batched AV epilogue via 32-row
col-tiled [v|0]/[ones32] matmuls, host-side output transpose.
"""

import sys

sys.path.insert(0, "/opt/trn_rl_repo")

import numpy as np
import ml_dtypes

import concourse.bass as bass
import concourse.tile as tile
from concourse import bacc, masks, mybir
from concourse.bass_utils import run_bass_kernel_spmd

BF16 = ml_dtypes.bfloat16
F32 = np.float32

N = 2048
C = 128
CP = 16
H = 8
D = 16
WH = 64
NCORES = 8
NQ = N // NCORES          # 256 queries per core
NW = NQ + 2 * WH          # 384 window positions per core (full-range k/v)
NWC = NW // 128           # 3 w-chunks for k/v
NQH = 2                   # query halves of 128
NSL = 4                   # (qh, wlc) slabs
EPS = 1e-5
NEG = -30000.0

dt = mybir.dt
AF = mybir.ActivationFunctionType
ALU = mybir.AluOpType
AX = mybir.AxisListType

WNAMES = [
    "w_gate", "w_skip", "wq_a", "wq_b", "wk_a", "wk_b", "wv",
    "wg_a", "wg_b", "wo_a", "wo_b", "w_og",
    "w_tgate", "w_tskip", "w_sw0", "w_sw1", "w_hd0", "w_hd1",
    "w_to0", "w_to1", "w_tog", "wblk_a", "wblk_b", "ones32",
]
VNAMES = ["gate_b", "bg_a", "bg_b", "bo", "og_b", "t_gate_b", "tog_b"]


def build_graph(consts, debug_taps=False):
    nc = bacc.Bacc(
        "TRN2",
        target_bir_lowering=False,
        debug=False,
        enable_asserts=False,
        num_devices=NCORES,
    )

    def inp(name, shape, dtype):
        return nc.dram_tensor(name, shape, dtype, kind="ExternalInput")

    scat_d = inp("scat", [128, 6 * 128], dt.bfloat16)      # (sp,asr) x 3 row-tiles
    spt_d = inp("spt", [128, NW], dt.bfloat16)            # sp transposed [C, NW]
    pair_w = inp("pair_w", [NSL, 128, CP * 128], dt.bfloat16)  # [wl,(cp,ql)]
    pk_a = inp("pk_a", [NQH, 128, 8 * 256], dt.bfloat16)  # [(qm8,cp),(qG,wl)]
    pk_b = inp("pk_b", [NQH, 128, 8 * 256], dt.bfloat16)
    winadd_d = inp("winadd", [128, NSL * 128], dt.bfloat16)
    wcat_d = inp("wcat", [128, len(WNAMES) * 128], dt.bfloat16)
    vcat_d = inp("vcat", [128, len(VNAMES)], dt.float32)

    out_ext = nc.dram_tensor("out", [C, NQ], dt.float32, kind="ExternalOutput")
    def tap(nm, ap):
        if not debug_taps:
            return
        t = nc.dram_tensor(nm, list(ap.shape), ap.dtype, kind="ExternalOutput")
        nc.sync.dma_start(out=t[:], in_=ap)

    c0 = consts["c0"]  # [H] python floats
    qsl = slice(WH, WH + NQ)

    with tile.TileContext(nc) as tc:
        with (
            nc.allow_low_precision(reason="bf16 compute; tolerance 2e-2"),
            tc.tile_pool(name="const", bufs=1) as constp,
            tc.tile_pool(name="wpool", bufs=1) as wpool,
            tc.tile_pool(name="sing", bufs=1) as sing,
            tc.tile_pool(name="stmp", bufs=3) as stmp,
            tc.tile_pool(name="pairp", bufs=1) as pairp,
            tc.tile_pool(name="sqp", bufs=2) as sqp,
            tc.tile_pool(name="zsp", bufs=1) as zsp,
            tc.tile_pool(name="bac", bufs=1) as bac,
            tc.tile_pool(name="pbig", bufs=2) as pbig,
            tc.tile_pool(name="mmp", bufs=2, space="PSUM") as mmp,
            tc.tile_pool(name="sall", bufs=1, space="PSUM") as sallp,
            tc.tile_pool(name="avp", bufs=2, space="PSUM") as avp,
            tc.tile_pool(name="trp", bufs=2, space="PSUM") as trp,
        ):
            # ---------------- packed input loads ----------------
            # slabs lead both HWDGE queues (stats = longest chain), scat and
            # wcat next (gate singles/PE), pk after.
            slab_all = pairp.tile([128, NSL * 2048], dt.bfloat16,
                                  tag="slab_all", name="slab_all")
            slab_t = [slab_all[:, s * 2048:(s + 1) * 2048] for s in range(NSL)]
            pka_t, pkb_t = [None] * NQH, [None] * NQH
            scat = wpool.tile([128, 6 * 128], dt.bfloat16, tag="scat")
            wcat = wpool.tile([128, len(WNAMES) * 128], dt.bfloat16, tag="wcat")
            vcat = wpool.tile([128, len(VNAMES)], dt.float32, tag="vcat")
            spt = wpool.tile([128, NW], dt.bfloat16, tag="spt")
            wadd = wpool.tile([128, NSL * 128], dt.bfloat16, tag="wadd")
            nc.scalar.dma_start(out=scat[:], in_=scat_d[:])
            nc.sync.dma_start(out=slab_t[2], in_=pair_w[2])
            nc.scalar.dma_start(out=slab_t[3], in_=pair_w[3])
            nc.sync.dma_start(out=slab_t[0], in_=pair_w[0])
            nc.scalar.dma_start(out=slab_t[1], in_=pair_w[1])
            nc.sync.dma_start(out=wcat[:], in_=wcat_d[:])
            nc.scalar.dma_start(out=vcat[:], in_=vcat_d[:])
            for qh in range(NQH):
                ta = pairp.tile([128, 2048], dt.bfloat16,
                                tag=f"pka{qh}", name=f"pka{qh}")
                nc.sync.dma_start(out=ta[:], in_=pk_a[qh])
                pka_t[qh] = ta
                tb = pairp.tile([128, 2048], dt.bfloat16,
                                tag=f"pkb{qh}", name=f"pkb{qh}")
                nc.sync.dma_start(out=tb[:], in_=pk_b[qh])
                pkb_t[qh] = tb
            nc.scalar.dma_start(out=spt[:], in_=spt_d[:])
            nc.scalar.dma_start(out=wadd[:], in_=winadd_d[:])

            # ---------------- constants ----------------
            zero_c = constp.tile([128, 1], dt.float32, tag="zero_c")
            nc.vector.memset(zero_c[:], 0.0)
            eps_c = constp.tile([128, 1], dt.float32, tag="eps_c")
            nc.vector.memset(eps_c[:], EPS)
            nc.const_aps.aps[(dt.float32, 0.0)] = zero_c[:]
            nc.const_aps.aps[(dt.float32, EPS)] = eps_c[:]
            ident = constp.tile([128, 128], dt.bfloat16)
            masks.make_identity(nc, ident[:])
            # PE warm-up: ~60 back-to-back dummy matmuls (shared stationary)
            # keep the HAM clock gate open through the input-DMA phase (PE
            # would otherwise start the real stream cold at 1.2 GHz).

            wsb = {k: wcat[:, i * 128:(i + 1) * 128] for i, k in enumerate(WNAMES)}
            vsb = {k: vcat[:, i:i + 1] for i, k in enumerate(VNAMES)}

            # ---------------- singles token-LN stats (DVE first) ----------
            sn_T = sing.tile([128, NW], dt.bfloat16, tag="sn_T")
            lna_T = sing.tile([128, NW], dt.bfloat16, tag="lna_T")
            ln_items = []
            for rt in range(NWC):
                for ti, dstT in ((0, sn_T), (1, lna_T)):
                    xt = scat[:, (rt * 2 + ti) * 128:(rt * 2 + ti + 1) * 128]
                    mv = stmp.tile([128, 2], dt.float32, tag="ln_mv", bufs=6)
                    stats = stmp.tile([128, 6], dt.float32, tag="ln_stats")
                    nc.vector.bn_stats(out=stats[:], in_=xt)
                    nc.vector.bn_aggr(out=mv[:], in_=stats[:])
                    ln_items.append((rt, xt, dstT, mv))

            # ---------------- pair LN stats: fused squares ----------------
            # slab_all [128, (s4, cp16, ql128)].  Upper cp-planes squared
            # first (DVE: slabs 0-1, ACT: slabs 2-3), then ONE strided
            # DMA-accum folds planes [0,8) into [8,16) for all slabs while
            # the lower halves are squared.
            sq_all = sqp.tile([128, NSL * 2048], dt.bfloat16, tag="sq_all",
                              bufs=1, name="sq_all")

            def slab4(t, off, nsl=4, width=1024):
                tb = t if isinstance(t, bass.AP) else t[:]
                return bass.AP(tensor=tb.tensor, offset=tb.offset + off,
                               ap=[tb.ap[0], [2048, nsl], [1, width]])

            sab, sqb = slab_all[:], sq_all[:]
            nc.vector.tensor_tensor(
                slab4(sqb, 1024, 2), slab4(sab, 1024, 2), slab4(sab, 1024, 2),
                op=ALU.mult)
            nc.gpsimd.tensor_tensor(slab4(sqb, 4096 + 1024, 2),
                                    slab4(sab, 4096 + 1024, 2),
                                    slab4(sab, 4096 + 1024, 2), op=ALU.mult)
            nc.gpsimd.dma_start(out=slab4(sab, 1024), in_=slab4(sab, 0),
                                accum_op=ALU.add)
            nc.vector.tensor_tensor(
                slab4(sqb, 0, 2), slab4(sab, 0, 2), slab4(sab, 0, 2),
                op=ALU.mult)
            nc.gpsimd.tensor_tensor(slab4(sqb, 4096, 2), slab4(sab, 4096, 2),
                                    slab4(sab, 4096, 2), op=ALU.mult)

            # singles LN epilogue: sqrts clustered, then normalize+transpose
            ln_sds = []
            for rt, xt, dstT, mv in ln_items:
                sd = stmp.tile([128, 1], dt.float32, tag="ln_sd", bufs=6)
                nc.scalar.activation(sd[:], mv[:, 1:2], AF.Sqrt, bias=EPS)
                ln_sds.append(sd)
            for (rt, xt, dstT, mv), sd in zip(ln_items, ln_sds):
                rows = slice(rt * 128, (rt + 1) * 128)
                rsv = stmp.tile([128, 1], dt.float32, tag="ln_rs")
                nc.vector.reciprocal_approx_fast(rsv[:], sd[:])
                nmrs = stmp.tile([128, 1], dt.float32, tag="ln_nm")
                nc.vector.scalar_tensor_tensor(
                    nmrs[:], mv[:, 0:1], -1.0, rsv[:], ALU.mult, ALU.mult
                )
                lnx = stmp.tile([128, 128], dt.bfloat16, tag="ln_out")
                nb = nmrs[:]
                nmb = bass.AP(tensor=nb.tensor, offset=nb.offset,
                              ap=[nb.ap[0], [0, 128]])
                nc.vector.scalar_tensor_tensor(
                    lnx[:], xt, rsv[:], nmb, ALU.mult, ALU.add)
                tp = trp.tile([128, 1024], dt.bfloat16, tag="tr")
                nc.tensor.transpose(tp[:, 0:128], lnx[:], ident[:])
                nc.vector.tensor_copy(dstT[:, rows], tp[:, 0:128])

            def mm(wname, rhs_ap, n, tag="mm"):
                ps = mmp.tile([128, 512], dt.float32, tag=tag)
                nc.tensor.matmul(ps[:, 0:n], wsb[wname], rhs_ap, start=True, stop=True)
                return ps

            # AdaLN for attention branch
            g_ps = mm("w_gate", sn_T[:], NW)
            gate_sb = stmp.tile([128, NW], dt.bfloat16, tag="gate")
            nc.scalar.activation(gate_sb[:], g_ps[:, 0:NW], AF.Sigmoid, bias=vsb["gate_b"])
            sk_ps = mm("w_skip", sn_T[:], NW)
            tg1 = stmp.tile([128, NW], dt.bfloat16, tag="atg1")
            nc.vector.tensor_tensor(tg1[:], gate_sb[:], lna_T[:], op=ALU.mult)
            a_T = sing.tile([128, NW], dt.bfloat16, tag="a_T")
            nc.vector.tensor_tensor(a_T[:], tg1[:], sk_ps[:, 0:NW], op=ALU.add)

            q_Ts, k_Ts, gq_Ts = [], [], []
            for grp in ("a", "b"):
                q_ps = mm(f"wq_{grp}", a_T[:, qsl], NQ)
                q_Tg = sing.tile([128, NQ], dt.bfloat16, tag=f"q_T{grp}")
                nc.scalar.copy(q_Tg[:], q_ps[:, 0:NQ])  # D^-0.5 folded in wq
                q_Ts.append(q_Tg)
                k_ps = mm(f"wk_{grp}", a_T[:], NW)
                k_Tg = sing.tile([128, NW], dt.bfloat16, tag=f"k_T{grp}")
                nc.scalar.copy(k_Tg[:], k_ps[:, 0:NW])
                k_Ts.append(k_Tg)

            v_ps = mm("wv", a_T[:], NW)
            v_T = sing.tile([128, NW], dt.bfloat16, tag="v_T")  # tight (h,d)
            nc.scalar.copy(v_T[:], v_ps[:, 0:NW])

            # v_pad[wc]: [wl, (h: v16 | zero16)] via PE transpose + strided copy
            v_pad = []
            for wc in range(NWC):
                vtw = trp.tile([128, 1024], dt.bfloat16, tag="tr")
                nc.tensor.transpose(vtw[:, 0:128], v_T[:, wc * 128:(wc + 1) * 128],
                                    ident[:])
                vp = sing.tile([128, 256], dt.bfloat16, tag=f"vpad{wc}")
                nc.gpsimd.memset(vp[:], 0.0)
                vpb, tpb = vp[:], vtw[:, 0:128]
                dst = bass.AP(tensor=vpb.tensor, offset=vpb.offset,
                              ap=[vpb.ap[0], [32, H], [1, D]])
                src = bass.AP(tensor=tpb.tensor, offset=tpb.offset,
                              ap=[tpb.ap[0], [D, H], [1, D]])
                nc.vector.tensor_copy(dst, src)
                v_pad.append(vp)
            tap("d_vpad0", v_pad[0][:])
            tap("d_aT", a_T[:])

            # deferred late-needed gates (attention epilogue / final add)
            for grp in ("a", "b"):
                g2_ps = mm(f"wg_{grp}", a_T[:, qsl], NQ)
                gq_Tg = sing.tile([128, NQ], dt.bfloat16, tag=f"gq_T{grp}",
                                  name=f"gq_T{grp}")
                nc.scalar.activation(
                    gq_Tg[:], g2_ps[:, 0:NQ], AF.Sigmoid, bias=vsb[f"bg_{grp}"]
                )
                gq_Ts.append(gq_Tg)

            # out gates (raw sp projections, from host-transposed spt)
            og_ps = mm("w_og", spt[:, qsl], NQ)
            og_sb = stmp.tile([128, NQ], dt.bfloat16, tag="og")
            nc.scalar.activation(og_sb[:], og_ps[:, 0:NQ], AF.Sigmoid, bias=vsb["og_b"])
            tog_ps = mm("w_tog", spt[:, qsl], NQ)
            tog_sb = stmp.tile([128, NQ], dt.bfloat16, tag="tog")
            nc.scalar.activation(tog_sb[:], tog_ps[:, 0:NQ], AF.Sigmoid, bias=vsb["tog_b"])

            # transition branch
            tgp = mm("w_tgate", sn_T[:, qsl], NQ)
            tgate = stmp.tile([128, NQ], dt.bfloat16, tag="tgate")
            nc.scalar.activation(tgate[:], tgp[:, 0:NQ], AF.Sigmoid, bias=vsb["t_gate_b"])
            tskp = mm("w_tskip", sn_T[:, qsl], NQ)
            tt1 = stmp.tile([128, NQ], dt.bfloat16, tag="tt1")
            nc.vector.tensor_tensor(tt1[:], tgate[:], lna_T[:, qsl], op=ALU.mult)
            t_T = sing.tile([128, NQ], dt.bfloat16, tag="t_T")
            nc.vector.tensor_tensor(t_T[:], tt1[:], tskp[:, 0:NQ], op=ALU.add)

            hid = []
            for half in range(2):
                swp = mm(f"w_sw{half}", t_T[:], NQ)
                sg = stmp.tile([128, NQ], dt.bfloat16, tag=f"sg{half}")
                nc.scalar.activation(sg[:], swp[:, 0:NQ], AF.Sigmoid)
                sw = stmp.tile([128, NQ], dt.bfloat16, tag=f"sw{half}")
                nc.vector.tensor_tensor(sw[:], sg[:], swp[:, 0:NQ], op=ALU.mult)
                hdp = mm(f"w_hd{half}", t_T[:], NQ)
                hh = sing.tile([128, NQ], dt.bfloat16, tag=f"hid{half}")
                nc.vector.tensor_tensor(hh[:], sw[:], hdp[:, 0:NQ], op=ALU.mult)
                hid.append(hh)
            tr_ps = mmp.tile([128, 512], dt.float32, tag="mm")
            nc.tensor.matmul(tr_ps[:, 0:NQ], wsb["w_to0"], hid[0][:], start=True, stop=False)
            nc.tensor.matmul(tr_ps[:, 0:NQ], wsb["w_to1"], hid[1][:], start=False, stop=True)
            trans_g = sing.tile([128, NQ], dt.float32, tag="trans_g")
            nc.vector.tensor_tensor(trans_g[:], tog_sb[:], tr_ps[:, 0:NQ], op=ALU.mult)

            # ---------------- pair LN stats: fused trees ------------------
            hp_ctx = tc.high_priority()
            hp_ctx.__enter__()
            sl1 = sqp.tile([128, 4096], dt.bfloat16, tag="sl1", bufs=1)
            sl1b = sl1[:]
            nc.vector.tensor_tensor(
                bass.AP(tensor=sl1b.tensor, offset=sl1b.offset,
                        ap=[sl1b.ap[0], [1024, 4], [1, 1024]]),
                slab4(sqb, 0), slab4(sqb, 1024), op=ALU.add)
            l2q = sqp.tile([128, 2048], dt.bfloat16, tag="l2q", bufs=1)
            l2qb = l2q[:]

            def lev(t, stride, off, width):
                tb = t if isinstance(t, bass.AP) else t[:]
                return bass.AP(tensor=tb.tensor, offset=tb.offset + off,
                               ap=[tb.ap[0], [stride, 4], [1, width]])

            nc.vector.tensor_tensor(lev(l2qb, 512, 0, 512),
                                    lev(sl1b, 1024, 0, 512),
                                    lev(sl1b, 1024, 512, 512), op=ALU.add)
            l3q = sqp.tile([128, 1024], dt.bfloat16, tag="l3q", bufs=1)
            l3qb = l3q[:]
            nc.vector.tensor_tensor(lev(l3qb, 256, 0, 256),
                                    lev(l2qb, 512, 0, 256),
                                    lev(l2qb, 512, 256, 256), op=ALU.add)
            sumsq_all = stmp.tile([128, 512], dt.float32, tag="sumsq_all",
                                  bufs=1)
            nc.vector.tensor_tensor(sumsq_all[:],
                                    lev(l3qb, 256, 0, 128),
                                    lev(l3qb, 256, 128, 128), op=ALU.add)
            # raw sums tail: planes 8..16 hold pairwise sums after the L1 DMA
            rl2 = sqp.tile([128, 2048], dt.bfloat16, tag="rl2", bufs=1)
            rl2b = rl2[:]
            nc.vector.tensor_tensor(lev(rl2b, 512, 0, 512),
                                    slab4(sab, 1024, 4, 512),
                                    slab4(sab, 1536, 4, 512), op=ALU.add)
            rl3 = sqp.tile([128, 1024], dt.bfloat16, tag="rl3", bufs=1)
            rl3b = rl3[:]
            nc.vector.tensor_tensor(lev(rl3b, 256, 0, 256),
                                    lev(rl2b, 512, 0, 256),
                                    lev(rl2b, 512, 256, 256), op=ALU.add)
            sums_all = stmp.tile([128, 512], dt.float32, tag="sums_all",
                                 bufs=1)
            nc.vector.tensor_tensor(sums_all[:],
                                    lev(rl3b, 256, 0, 128),
                                    lev(rl3b, 256, 128, 128), op=ALU.add)
            # var*16 = sumsq - sums^2/16; rs = 1/sqrt(var + eps)
            nm2 = stmp.tile([128, 512], dt.float32, tag="nm2", bufs=1)
            nc.vector.scalar_tensor_tensor(
                nm2[:], sums_all[:], -1.0 / CP, sums_all[:], ALU.mult, ALU.mult)
            var16 = stmp.tile([128, 512], dt.float32, tag="var16", bufs=1)
            nc.vector.tensor_tensor(var16[:], sumsq_all[:], nm2[:], op=ALU.add)
            varc = stmp.tile([128, 512], dt.float32, tag="varc", bufs=1)
            nc.vector.tensor_scalar_max(varc[:], var16[:], 0.0)
            sd_all = stmp.tile([128, 512], dt.float32, tag="sd_all", bufs=1)
            nc.scalar.activation(sd_all[:], varc[:], AF.Sqrt, bias=EPS,
                                 scale=1.0 / CP)
            rs_all = stmp.tile([128, 512], dt.float32, tag="rs_all", bufs=1)
            nc.vector.reciprocal_approx_fast(rs_all[:], sd_all[:])
            hp_ctx.__exit__(None, None, None)
            tap("d_sums0", sums_all[:, 0:128])
            tap("d_sumsq0", sumsq_all[:, 0:128])
            tap("d_rs0", rs_all[:, 0:128])

            # ---------------- z path ----------------
            # z[(h,qm), (qG, wl)] per qh; evac to bf16; xbar-transpose blocks
            # into bacc[(qh,wlc)][wl, (qG, h, qm)]; scale by rs broadcast.
            # single bias tile, cols = (s=qh*2+wlc)*1024 + qG*128 + h*16 + qm
            bacc_t = bac.tile([128, 4096], dt.bfloat16, tag="bacc", name="bacc")
            for qh in range(NQH):
                zs = zsp.tile([128, 2048], dt.bfloat16, tag=f"zs{qh}")
                for ch in range(4):
                    csl = slice(ch * 512, (ch + 1) * 512)
                    zp = mmp.tile([128, 512], dt.float32, tag="mm")
                    nc.tensor.matmul(zp[:], wsb["wblk_a"], pka_t[qh][:, csl],
                                     start=True, stop=False)
                    nc.tensor.matmul(zp[:], wsb["wblk_b"], pkb_t[qh][:, csl],
                                     start=False, stop=True)
                    if ch % 2 == 0:
                        nc.scalar.copy(zs[:, csl], zp[:])
                    else:
                        nc.vector.tensor_copy(zs[:, csl], zp[:])
                if qh == 0:
                    tap("d_zs0", zs[:])
                # PE-transpose (qG, wlc) blocks, 4 per trp tile, then one
                # 512-col scale-by-rs TT per group while evacuating PSUM.
                for wlc in range(2):
                    s = qh * 2 + wlc
                    for qg4 in range(2):
                        tp = trp.tile([128, 1024], dt.bfloat16, tag="tr")
                        for k in range(4):
                            qg = qg4 * 4 + k
                            src = zs[:, qg * 256 + wlc * 128:
                                     qg * 256 + (wlc + 1) * 128]
                            nc.tensor.transpose(
                                tp[:, k * 128:(k + 1) * 128], src, ident[:])
                        tb = tp[:]
                        t4 = bass.AP(tensor=tb.tensor, offset=tb.offset,
                                     ap=[tb.ap[0], [128, 4], [16, 8], [1, 16]])
                        bb = bacc_t[:]
                        b4 = bass.AP(
                            tensor=bb.tensor,
                            offset=bb.offset + s * 1024 + qg4 * 512,
                            ap=[bb.ap[0], [128, 4], [16, 8], [1, 16]])
                        rb = rs_all[:]
                        r4 = bass.AP(tensor=rb.tensor,
                                     offset=rb.offset + s * 128 + qg4 * 64,
                                     ap=[rb.ap[0], [16, 4], [0, 8], [1, 16]])
                        nc.vector.tensor_tensor(b4, t4, r4, op=ALU.mult)
                if qh == 0:
                    tap("d_bacc0", bacc_t[:, 0:1024])

            # ---------------- attention ----------------
            s_all = sallp.tile([128, 1024], dt.float32, tag="s_all")  # 2 banks
            for _ in range(25):
                nc.tensor.matmul(s_all[:, 0:128], ident[:], ident[:],
                                 start=True, stop=True, skip_group_check=True)
            # 4 score slots (2 in s_all + 2 borrowed from the idle mm pool)
            # so the PE pipeline runs 4 heads deep ahead of the exps.
            sx0 = mmp.tile([128, 512], dt.float32, tag="mm")
            sx1 = mmp.tile([128, 512], dt.float32, tag="mm")
            slots = [s_all[:, 0:512], s_all[:, 512:1024], sx0[:], sx1[:]]
            # per head: scores for BOTH query halves in one [128, 512] slot,
            # cols = (s = qh*2+wlc)*128 + ql.  AV matmuls issued inline two
            # heads behind to keep the PE busy during the exps.
            avs = [avp.tile([128, 512], dt.float32, tag="av", name=f"av{qh}")
                   for qh in range(NQH)]
            pbs = []
            t2g = [sing.tile([128, NQ], dt.bfloat16, tag=f"t2g{g}",
                             name=f"t2g{g}") for g in range(2)]

            def epi(qh, grp):
                av = avs[qh]
                rden = stmp.tile([128, 128], dt.float32, tag="rden", bufs=4)
                nc.vector.reciprocal_approx_fast(
                    rden[:], av[:, grp * 256 + 128: grp * 256 + 256])
                u = stmp.tile([128, 128], dt.bfloat16, tag="u", bufs=4)
                nc.vector.tensor_tensor(
                    u[:], av[:, grp * 256: grp * 256 + 128], rden[:],
                    op=ALU.mult)
                nc.vector.tensor_tensor(
                    t2g[grp][:, qh * 128:(qh + 1) * 128], u[:],
                    gq_Ts[grp][:, qh * 128:(qh + 1) * 128], op=ALU.mult)

            def av_mms(h):
                grp, hb = h // 4, (h % 4) * 32
                for s in range(4):
                    qh, wlc = s // 2, s % 2
                    wc = qh + wlc
                    rhs = pbs[h][:, s * 128:(s + 1) * 128]
                    # start=True on the first MM of each 32-row block: marks
                    # that partition range's whole bank pending-zero
                    nc.tensor.matmul(
                        avs[qh][hb:hb + 32, grp * 256: grp * 256 + 128],
                        v_pad[wc][:, h * 32:(h + 1) * 32], rhs,
                        start=(wlc == 0 and h < 4), stop=False,
                        tile_position=(0, hb), skip_group_check=True)
                    nc.tensor.matmul(
                        avs[qh][hb:hb + 32, grp * 256 + 128: grp * 256 + 256],
                        wsb["ones32"][:, 0:32], rhs,
                        start=False, stop=(wlc == 1 and h % 4 == 3),
                        tile_position=(0, hb), skip_group_check=True)

            for h in range(H):
                st = slots[h % 4]
                bb = bacc_t[:]
                rhs = bass.AP(tensor=bb.tensor, offset=bb.offset + h * 16,
                              ap=[bb.ap[0], [1024, 4], [128, 8], [1, 16]])
                nc.tensor.matmul(st, ident[:], rhs, start=True, stop=False,
                                 skip_group_check=True)
                nc.tensor.matmul(st, ident[:], wadd[:, 0:512],
                                 start=False, stop=False, skip_group_check=True)
                grp, hb = h // 4, (h % 4) * 32
                h32 = slice(hb, hb + 32)
                # k-chunk wc serves (qh, wlc) pairs with qh+wlc == wc
                for wc, (oc, q0c, qn) in enumerate(
                        ((0, 0, 128), (128, 0, 256), (384, 128, 128))):
                    nc.tensor.matmul(
                        st[:, oc:oc + qn],
                        k_Ts[grp][h32, wc * 128:(wc + 1) * 128],
                        q_Ts[grp][h32, q0c:q0c + qn],
                        start=False, stop=(wc == 2),
                        tile_position=(hb, 0), skip_group_check=True)
                pb = pbig.tile([128, 512], dt.bfloat16, tag=f"pb{h}")
                nc.scalar.activation(pb[:], st, AF.Exp, bias=float(c0[h]))
                pbs.append(pb)
                if h == 0:
                    tap("d_pb0", pb[:])
                if h >= 2:
                    av_mms(h - 2)
                if h == H - 1:
                    # grp0 (heads 0-3, av cols 0:256) is complete after
                    # av_mms(3): run its epilogue while grp1 AV continues
                    for qh in range(NQH):
                        epi(qh, 0)
            av_mms(H - 2)
            av_mms(H - 1)
            for qh in range(NQH):
                epi(qh, 1)
            ao_ps = mmp.tile([128, 512], dt.float32, tag="mm")
            nc.tensor.matmul(ao_ps[:, 0:NQ], wsb["wo_a"], t2g[0][:],
                             start=True, stop=False, skip_group_check=True)
            nc.tensor.matmul(ao_ps[:, 0:NQ], wsb["wo_b"], t2g[1][:],
                             start=False, stop=True, skip_group_check=True)
            attn = stmp.tile([128, NQ], dt.float32, tag="attn")
            nc.vector.scalar_tensor_tensor(
                attn[:], ao_ps[:, 0:NQ], vsb["bo"], og_sb[:], ALU.add, ALU.mult)
            fin = stmp.tile([128, NQ], dt.float32, tag="fin")
            nc.vector.tensor_tensor(fin[:], attn[:], trans_g[:], op=ALU.add)
            nc.sync.dma_start(out=out_ext[:], in_=fin[:])

    nc.finalize()
    return nc


def _prep(inputs):
    """Host-side shard + pack. Returns (in_maps, consts)."""
    f = {k: np.asarray(v) for k, v in inputs.items()}
    pair = f["atom_pair_repr"][0]          # [N, N, CP]
    asr = f["atom_single_repr"][0]         # [N, C]
    sp = f["atom_single_proj"][0]
    mask = f["mask"][0]                    # [N]

    ln_scale = f["pair_ln_scale"]
    ln_bias = f["pair_ln_bias"]
    wp_s = ln_scale[:, None] * f["w_pair"]           # [CP, H] scaled
    c0 = (ln_bias @ f["w_pair"]).astype(np.float64)  # [H]
    wp = wp_s - wp_s.sum(0, keepdims=True) / CP      # fold mean subtraction

    s_scale = f["adaln_s_scale"]
    t_scale = f["t_s_scale"]

    def bf(x):
        return np.ascontiguousarray(x.astype(BF16))

    def pad_heads(w, scale=1.0):
        a = np.zeros((C, 128), F32)
        b = np.zeros((C, 128), F32)
        for h4 in range(4):
            a[:, h4 * 32: h4 * 32 + D] = w[:, h4 * D:(h4 + 1) * D] * scale
            b[:, h4 * 32: h4 * 32 + D] = w[:, (h4 + 4) * D:(h4 + 5) * D] * scale
        return bf(a), bf(b)

    wq_a, wq_b = pad_heads(f["wq"], float(D) ** -0.5)
    wk_a, wk_b = pad_heads(f["wk"])
    wg_a, wg_b = pad_heads(f["wg"])

    def pad_rows(w):
        a = np.zeros((128, C), F32)
        b = np.zeros((128, C), F32)
        for h4 in range(4):
            a[h4 * 32: h4 * 32 + D, :] = w[h4 * D:(h4 + 1) * D, :]
            b[h4 * 32: h4 * 32 + D, :] = w[(h4 + 4) * D:(h4 + 5) * D, :]
        return bf(a), bf(b)

    wo_a, wo_b = pad_rows(f["wo"])
    bg_pad = np.zeros((2, 128), F32)
    for h4 in range(4):
        bg_pad[0, h4 * 32: h4 * 32 + D] = f["bg"][h4 * D:(h4 + 1) * D]
        bg_pad[1, h4 * 32: h4 * 32 + D] = f["bg"][(h4 + 4) * D:(h4 + 5) * D]

    # wblk: rows (qm_s 8, cp 16) -> cols h-major (h*16 + qm)
    wblk_a = np.zeros((128, 128), F32)
    wblk_b = np.zeros((128, 128), F32)
    for qms in range(8):
        for h in range(H):
            wblk_a[qms * 16: qms * 16 + 16, h * 16 + qms] = wp[:, h]
            wblk_b[qms * 16: qms * 16 + 16, h * 16 + 8 + qms] = wp[:, h]

    ones32 = np.zeros((128, 128), F32)
    ones32[:, 0:32] = 1.0

    weights = {
        "wq_a": wq_a, "wq_b": wq_b, "wk_a": wk_a, "wk_b": wk_b,
        "wg_a": wg_a, "wg_b": wg_b, "wo_a": wo_a, "wo_b": wo_b,
        "w_gate": bf(s_scale[:, None] * f["adaln_gate_w"]),
        "w_skip": bf(s_scale[:, None] * f["adaln_skip_w"]),
        "wv": bf(f["wv"]),
        "w_og": bf(f["out_gate_w"]),
        "w_tgate": bf(t_scale[:, None] * f["t_gate_w"]),
        "w_tskip": bf(t_scale[:, None] * f["t_skip_w"]),
        "w_sw0": bf(f["t_swish_w"][:, :128]), "w_sw1": bf(f["t_swish_w"][:, 128:]),
        "w_hd0": bf(f["t_hidden_w"][:, :128]), "w_hd1": bf(f["t_hidden_w"][:, 128:]),
        "w_to0": bf(f["t_out_w"][:128, :]), "w_to1": bf(f["t_out_w"][128:, :]),
        "w_tog": bf(f["t_out_gate_w"]),
        "wblk_a": bf(wblk_a), "wblk_b": bf(wblk_b), "ones32": bf(ones32),
    }
    wcat = np.concatenate([weights[k] for k in WNAMES], axis=1)
    vecs = {
        "gate_b": f["adaln_gate_b"], "bg_a": bg_pad[0], "bg_b": bg_pad[1],
        "bo": f["bo"],
        "og_b": f["out_gate_b"], "t_gate_b": f["t_gate_b"],
        "tog_b": f["t_out_gate_b"],
    }
    vcat = np.stack([vecs[k].astype(F32).reshape(128) for k in VNAMES], axis=1)

    shared = {
        "wcat": np.ascontiguousarray(wcat),
        "vcat": np.ascontiguousarray(vcat),
    }

    in_maps = []
    for i in range(NCORES):
        i0 = i * NQ
        lo = i0 - WH
        ks, ke = max(lo, 0), min(i0 + NQ + WH, N)
        # halo singles rows (for on-device LN) + transposed raw sp
        halo = np.zeros((NW, C), F32)
        halo_s = np.zeros((NW, C), F32)
        halo[ks - lo: ke - lo] = asr[ks:ke]
        halo_s[ks - lo: ke - lo] = sp[ks:ke]
        scat = np.concatenate(
            [x for rt in range(NWC)
             for x in (halo_s[rt * 128:(rt + 1) * 128],
                       halo[rt * 128:(rt + 1) * 128])],
            axis=1,
        )
        spt = bf(halo_s.T)

        # pair slabs / pk packs, windowed per query half
        slabs = np.zeros((NSL, 128, CP * 128), F32)
        pk_a_i = np.zeros((NQH, 128, 2048), F32)
        pk_b_i = np.zeros((NQH, 128, 2048), F32)
        winadd_i = np.zeros((128, NSL * 128), F32)
        qidx = np.arange(128)
        for qh in range(NQH):
            q0 = i0 + qh * 128
            w0 = q0 - WH
            # strip [ql, wl, cp] for wl in [0, 256)
            strip = np.zeros((128, 256, CP), F32)
            ws, we = max(w0, 0), min(w0 + 256, N)
            strip[:, ws - w0: we - w0] = pair[q0: q0 + 128, ws:we]
            for wlc in range(2):
                s = qh * 2 + wlc
                blk = strip[:, wlc * 128:(wlc + 1) * 128]    # [ql, wl, cp]
                slabs[s] = blk.transpose(1, 2, 0).reshape(128, CP * 128)
                wabs = w0 + wlc * 128 + qidx
                inb = (wabs >= 0) & (wabs < N)
                mstrip = np.where(inb, mask[np.clip(wabs, 0, N - 1)], 0.0)
                valid = (
                    (np.abs(wabs[:, None] - (q0 + qidx)[None, :]) <= WH)
                    & inb[:, None] & (mstrip[:, None] > 0.5)
                )
                winadd_i[:, s * 128:(s + 1) * 128] = np.where(valid, 0.0, NEG)
            s4 = strip.reshape(8, 16, 256, CP)               # [qG, qm, wl, cp]
            for half, dst in ((0, pk_a_i), (1, pk_b_i)):
                t = s4[:, half * 8: half * 8 + 8]            # [qG, 8, wl, cp]
                t = t.transpose(1, 3, 0, 2)                  # [qm, cp, qG, wl]
                dst[qh] = t.reshape(128, 2048)

        in_maps.append({
            **shared,
            "scat": bf(scat),
            "spt": spt,
            "pair_w": bf(slabs),
            "pk_a": bf(pk_a_i),
            "pk_b": bf(pk_b_i),
            "winadd": bf(winadd_i),
        })
    return in_maps, {"c0": c0}


_CACHE = {}


def kernel(**inputs):
    in_maps, consts = _prep(inputs)
    key = "graph"
    if key not in _CACHE:
        _CACHE[key] = build_graph(consts)
    nc = _CACHE[key]
    res = run_bass_kernel_spmd(nc, in_maps, core_ids=list(range(NCORES)))
    out = np.concatenate(
        [res.results[i]["out"].T for i in range(NCORES)], axis=0)
    return out.reshape(1, N, C).astype(np.float32)


if __name__ == "__main__":
    import reference

    ins = reference.setup_inputs()
    ins = {k: np.asarray(v) for k, v in ins.items()}
    got = kernel(**ins)
    exp = np.asarray(reference.reference(**reference.setup_inputs()))
    err = np.abs(got - exp).max() / (np.abs(exp).max() + 1e-9)
    print("Relative error:", err)
